# revision 25
# baseline (speedup 1.0000x reference)
"""ConvCapsule Trainium2 kernel.

Full inputs -> 8-way batch-parallel (over output batch b) -> full output.

Math (per core, b = core id):
  img j in 0..7:  votes[j] = conv3x3_SAME(x[j, :, :, b, :], W)  -> [32,32,256]
  preact1 = (1/16) * sum_j votes[j] + bias          (softmax of zero logits = 1/16)
  act1    = squash(preact1)   [squash over dc groups of 16]
  logits[j, s, nc] = sum_dc votes[j][s, nc, dc] * act1[s, nc, dc]
  route   = softmax(logits over nc)
  preact2 = sum_j route[j] * votes[j] + bias
  out     = squash(preact2)

The end-to-end call is dominated by the axon-tunnel transfer (~30 MB/s each
way), so host<->device traffic is minimized:
  - per-core x slice ships as fp16 [128, 1024] (img*ch channel-major); the
    im2col expansion happens ON DEVICE via strided SBUF->SBUF DMAs instead
    of shipping a 4 MB/core S tensor.
  - the packed conv weight tables (identical on every core) are uploaded
    once and cached on device, keyed by content hash of (W, b).
  - output is int8 with fixed scale 127 (squash output is always in
    (-1, 1)), dequantized on host; halves the readback vs fp16.
  - the jax.jit(shard_map) executable is built once and cached; donated
    output buffers are created on-device (no zero upload per call).

Device mapping (per core):
  - conv as 2 accumulated matmuls (K=96/97 + K=48) per 128-pixel chunk per
    image, reading 6 w/h-shifted channel groups from the on-device-built S
    tile (zero padded, ones row for fused bias).
  - preact1 via duplicate matmuls with W/16-scaled weights accumulating in
    PSUM.
  - routing on DVE/ACT/GPSIMD; squash factor applied after the grouped
    reduce (linearity); output quantization scale folded into the final
    squash factor.
"""

import hashlib
from concurrent.futures import ThreadPoolExecutor

import numpy as np

import jax
import jax.numpy as jnp
from jax.sharding import Mesh, NamedSharding, PartitionSpec
from jax.experimental.shard_map import shard_map

import concourse.bacc as bacc
import concourse.tile as tile
from concourse import mybir
from concourse import bass2jax

F32 = mybir.dt.float32
F16 = mybir.dt.float16
I8 = mybir.dt.int8
NP_F16 = np.float16
AF = mybir.ActivationFunctionType
OP = mybir.AluOpType

B, H, W_, NIN, DIN = 8, 32, 32, 8, 16
NC, DC = 16, 16
O = NC * DC           # 256 out channels
SF = 36 * 32          # S free dim: 34 zero-padded rows of 32, + 2 rows tail
EPS = 1e-9
NCHUNK = 8            # spatial chunks of 128 pixels (4 rows)
NCORES = 8
GROUP_SIZES = (4, 4)    # cores per sub-mesh launch group (pipelining)
ASYNC_GATHER = True     # issue copy_to_host_async right after launch
OSCALE = 127.0        # int8 output quantization scale
GPSIMD_DMULTS = 4     # how many of the 8 route*votes products go to GPSIMD
GPSIMD_BMULT = True   # B-product on gpsimd

# packed weight rows in the 256-wide view: wc96 | wc48 | wc96s | wc48s | b
WR96, WR48, WR96S, WR48S, WRB = 0, 96, 144, 241, 289
WROWS = 290                          # total packed rows
WSLAB = (WROWS * O + 1023) // 1024   # 73 rows of 1024
SHIFTS = [(-1, -1), (-1, 0), (-1, 1), (0, -1), (0, 0), (0, 1)]

_CACHE = {}


def build_module():
    nc = bacc.Bacc("TRN2", target_bir_lowering=False, debug=False)

    xin = nc.dram_tensor("xin", [128, 1024], F16, kind="ExternalInput")
    wsl = nc.dram_tensor("wsl", [WSLAB, 1024], F16, kind="ExternalInput")
    out = nc.dram_tensor("out", [H * W_, O], I8, kind="ExternalOutput")
    # 256-wide view of the packed weight slab
    wv = wsl.ap().rearrange("p (q r) -> (p q) r", r=O)

    with tile.TileContext(nc) as tc:
        with (
            tc.tile_pool(name="const", bufs=1) as constp,
            tc.tile_pool(name="simg", bufs=1) as sp,
            tc.tile_pool(name="psum", bufs=1, space="PSUM") as pp,
            tc.tile_pool(name="work", bufs=2) as wp,
            tc.tile_pool(name="small", bufs=2) as smp,
        ):
            # ---- persistent loads ----
            w96 = constp.tile([96, O], F16)
            w48 = constp.tile([48, O], F16)
            w96s = constp.tile([97, O], F16)
            w48s = constp.tile([48, O], F16)
            brow = constp.tile([1, O], F16)
            nc.sync.dma_start(w96[:], wv[WR96:WR96 + 96])
            nc.sync.dma_start(w48[:], wv[WR48:WR48 + 48])
            nc.sync.dma_start(w96s[:], wv[WR96S:WR96S + 97])
            nc.sync.dma_start(w48s[:], wv[WR48S:WR48S + 48])
            nc.sync.dma_start(brow[:], wv[WRB:WRB + 1])

            xall = sp.tile([128, 1024], F16, name="xall")
            nc.sync.dma_start(xall[:], xin.ap())

            # bias tile [128, O] = ones[128]^T (x) brow, via PE broadcast
            ones1 = constp.tile([1, 128], F16)
            nc.vector.memset(ones1[:], 1.0)
            ps_b = pp.tile([128, O], F32, tag="psb", bufs=1)
            nc.tensor.matmul(ps_b[:], ones1[:], brow[:], start=True, stop=True)
            bias = constp.tile([128, O], F32)
            nc.scalar.copy(bias[:], ps_b[:])

            # ---- on-device im2col: S[j][16g+ch, r*32+w] = xpad[j,ch,r-1+dh,w+dw]
            s_tiles = []
            for j in range(NIN):
                st = sp.tile([97, SF], F16, name=f"s{j}")
                nc.vector.memset(st[0:96, :], 0.0)
                nc.gpsimd.memset(st[96:97, :], 1.0)
                s_tiles.append(st)
            xv = xall[:].rearrange("p (h w) -> p h w", w=32)
            for j in range(NIN):
                sv = s_tiles[j][:].rearrange("p (r w) -> p r w", w=32)
                for g, (dh, dw) in enumerate(SHIFTS):
                    rlo, rhi = max(0, 1 - dh), min(34, 33 - dh)
                    wlo, whi = max(0, -dw), min(32, 32 - dw)
                    nc.sync.dma_start(
                        sv[16 * g:16 * g + 16, rlo:rhi, wlo:whi],
                        xv[j * 16:(j + 1) * 16,
                           rlo - 1 + dh:rhi - 1 + dh, wlo + dw:whi + dw])

            for c in range(NCHUNK):
                h0 = 4 * c
                # ---------------- conv ----------------
                ps_votes = pp.tile([128, NIN * O], F32, tag="psv", bufs=1)
                ps_pre1 = pp.tile([128, O], F32, tag="psp", bufs=1)
                p0 = (h0 + 1) * 32
                for j in range(NIN):
                    st = s_tiles[j]
                    l96 = st[0:96, p0:p0 + 128]
                    l97 = st[0:97, p0:p0 + 128]
                    l48 = st[0:48, p0 + 64:p0 + 192]
                    vslice = ps_votes[:, j * O:(j + 1) * O]
                    nc.tensor.matmul(vslice, l96, w96[:], start=True, stop=False,
                                     skip_group_check=True)
                    if j == 0:
                        nc.tensor.matmul(ps_pre1[:], l97, w96s[:],
                                         start=True, stop=False,
                                         skip_group_check=True)
                    else:
                        nc.tensor.matmul(ps_pre1[:], l96, w96s[0:96],
                                         start=False, stop=False,
                                         skip_group_check=True)
                    nc.tensor.matmul(vslice, l48, w48[:], start=False, stop=True,
                                     skip_group_check=True)
                    nc.tensor.matmul(ps_pre1[:], l48[0:48], w48s[:],
                                     start=False, stop=(j == NIN - 1),
                                     skip_group_check=True)

                # ---------------- evict ----------------
                votes = wp.tile([128, NIN * O], F32, tag="votes")
                pre1 = smp.tile([128, O], F32, tag="pre1")
                nc.scalar.copy(votes[:], ps_votes[:])
                nc.scalar.copy(pre1[:], ps_pre1[:])

                # ---------------- squash factor f1 from preact1 ----------------
                sqel1 = smp.tile([128, O], F32, tag="sqel1")
                nc.scalar.square(sqel1[:], pre1[:])
                sq1 = smp.tile([128, NC], F32, tag="sq1")
                nc.vector.reduce_sum(
                    sq1[:], sqel1[:].rearrange("p (n d) -> p n d", d=DC),
                    axis=mybir.AxisListType.X)
                f1 = _squash_factor(nc, smp, sq1, "1")

                # ---------------- logits ----------------
                pall = wp.tile([128, NIN * O], F32, tag="pall")
                v3 = votes[:].rearrange("p (j o) -> p j o", j=NIN)
                p1b = pre1[:].unsqueeze(1).broadcast_to([128, NIN, O])
                eng_b = nc.gpsimd if GPSIMD_BMULT else nc.vector
                eng_b.tensor_tensor(
                    pall[:].rearrange("p (j o) -> p j o", j=NIN), v3, p1b, op=OP.mult)
                lg = smp.tile([128, NIN * NC], F32, tag="lg")
                nc.vector.reduce_sum(
                    lg[:], pall[:].rearrange("p (j n d) -> p j n d", n=NC, d=DC),
                    axis=mybir.AxisListType.X)
                logits = smp.tile([128, NIN * NC], F32, tag="logits")
                f1b = f1[:].unsqueeze(1).broadcast_to([128, NIN, NC])
                nc.vector.tensor_tensor(
                    logits[:].rearrange("p (j n) -> p j n", j=NIN),
                    lg[:].rearrange("p (j n) -> p j n", j=NIN), f1b, op=OP.mult)

                # ---------------- softmax over nc ----------------
                ee = smp.tile([128, NIN * NC], F32, tag="ee")
                nc.scalar.activation(ee[:], logits[:], AF.Exp)
                den = smp.tile([128, NIN], F32, tag="den")
                nc.vector.reduce_sum(
                    den[:], ee[:].rearrange("p (j n) -> p j n", j=NIN),
                    axis=mybir.AxisListType.X)
                rcp = smp.tile([128, NIN], F32, tag="rcp")
                nc.vector.reciprocal(rcp[:], den[:])

                # ---------------- preact2 = sum_j route*votes + b ----------------
                route = smp.tile([128, NIN * NC], F32, tag="route")
                rcpb = rcp[:].unsqueeze(2).broadcast_to([128, NIN, NC])
                nc.vector.tensor_tensor(
                    route[:].rearrange("p (j n) -> p j n", j=NIN),
                    ee[:].rearrange("p (j n) -> p j n", j=NIN), rcpb, op=OP.mult)
                p2 = wp.tile([128, NIN * O], F32, tag="p2")
                for j in range(NIN):
                    rj = route[:, j * NC:(j + 1) * NC]
                    rjb = rj.unsqueeze(2).broadcast_to([128, NC, DC])
                    eng = nc.gpsimd if j < GPSIMD_DMULTS else nc.vector
                    eng.tensor_tensor(
                        p2[:, j * O:(j + 1) * O].rearrange("p (n d) -> p n d", n=NC),
                        votes[:, j * O:(j + 1) * O].rearrange("p (n d) -> p n d", n=NC),
                        rjb, op=OP.mult)
                pre2 = smp.tile([128, O], F32, tag="pre2")
                nc.vector.reduce_sum(
                    pre2[:],
                    p2[:].rearrange("p (j n d) -> p n d j", j=NIN, n=NC),
                    axis=mybir.AxisListType.X)
                pre2b = smp.tile([128, O], F32, tag="pre2b")
                nc.vector.tensor_tensor(pre2b[:], pre2[:], bias[:], op=OP.add)

                # ---------------- final squash, quantized to int8 ----------------
                sqel2 = smp.tile([128, O], F32, tag="sqel2")
                nc.scalar.square(sqel2[:], pre2b[:])
                sq2 = smp.tile([128, NC], F32, tag="sq2")
                nc.vector.reduce_sum(
                    sq2[:], sqel2[:].rearrange("p (n d) -> p n d", d=DC),
                    axis=mybir.AxisListType.X)
                f2 = _squash_factor(nc, smp, sq2, "2")
                f2s = smp.tile([128, NC], F32, tag="f2s")
                nc.vector.tensor_scalar_mul(f2s[:], f2[:], OSCALE)
                act2 = wp.tile([128, O], I8, tag="act2")
                f2b = f2s[:].unsqueeze(2).broadcast_to([128, NC, DC])
                nc.vector.tensor_tensor(
                    act2[:].rearrange("p (n d) -> p n d", n=NC),
                    pre2b[:].rearrange("p (n d) -> p n d", n=NC), f2b, op=OP.mult)

                nc.sync.dma_start(out.ap()[c * 128:(c + 1) * 128], act2[:])

    nc.compile()
    return nc


def _squash_factor(nc, pool, sq, tag):
    """f = sq / ((1+sq) * sqrt(sq+EPS)), shape [128, NC]."""
    sqe = pool.tile([128, NC], F32, name=f"sqe{tag}", tag=f"sqe{tag}")
    nc.vector.tensor_scalar_add(sqe[:], sq[:], EPS)
    rt = pool.tile([128, NC], F32, name=f"rt{tag}", tag=f"rt{tag}")
    nc.scalar.activation(rt[:], sqe[:], AF.Sqrt)
    u = pool.tile([128, NC], F32, name=f"u{tag}", tag=f"u{tag}")
    nc.vector.tensor_scalar_add(u[:], sq[:], 1.0)
    w = pool.tile([128, NC], F32, name=f"w{tag}", tag=f"w{tag}")
    nc.vector.tensor_tensor(w[:], u[:], rt[:], op=OP.mult)
    vr = pool.tile([128, NC], F32, name=f"vr{tag}", tag=f"vr{tag}")
    nc.vector.reciprocal(vr[:], w[:])
    f = pool.tile([128, NC], F32, name=f"f{tag}", tag=f"f{tag}")
    nc.vector.tensor_tensor(f[:], sq[:], vr[:], op=OP.mult)
    return f


def make_x_input(x):
    """[core, img, ch, h, w] fp16, flattened to the global [8*128, 1024]."""
    x = np.asarray(x)
    # transpose view + astype = one-pass permute-and-cast
    return x.transpose(3, 0, 4, 1, 2).astype(NP_F16).reshape(
        NCORES * 128, 1024)


def make_w_slab(W, b):
    """Packed weight tables, tiled per core: [8*73, 1024] fp16."""
    W = np.asarray(W, dtype=np.float32)
    b = np.asarray(b, dtype=np.float32)
    wpack = np.zeros((WROWS, O), np.float32)
    for g in range(6):
        kh, kw = (0, g) if g < 3 else (1, g - 3)
        wpack[WR96 + 16 * g:WR96 + 16 * g + 16] = W[kh, kw]
    for g in range(3):
        wpack[WR48 + 16 * g:WR48 + 16 * g + 16] = W[2, g]
    bflat = b.reshape(O)
    wpack[WR96S:WR96S + 96] = wpack[WR96:WR96 + 96] / 16.0
    wpack[WR96S + 96] = bflat
    wpack[WR48S:WR48S + 48] = wpack[WR48:WR48 + 48] / 16.0
    wpack[WRB] = bflat

    wslab = np.zeros((WSLAB, 1024), NP_F16)
    wslab.reshape(-1)[:WROWS * O] = wpack.reshape(-1).astype(NP_F16)
    return np.tile(wslab, (NCORES, 1))


def _get_runner():
    if "runner" in _CACHE:
        return _CACHE["runner"]

    nc = _CACHE.get("nc")
    if nc is None:
        nc = _CACHE["nc"] = build_module()

    bass2jax.install_neuronx_cc_hook()
    partition_name = nc.partition_id_tensor.name if nc.partition_id_tensor else None

    in_names, out_names, out_avals = [], [], []
    for alloc in nc.m.functions[0].allocations:
        if not isinstance(alloc, mybir.MemoryLocationSet):
            continue
        name = alloc.memorylocations[0].name
        if alloc.kind == "ExternalInput":
            if name != partition_name:
                in_names.append(name)
        elif alloc.kind == "ExternalOutput":
            out_names.append(name)
            out_avals.append(jax.core.ShapedArray(
                tuple(alloc.tensor_shape), mybir.dt.np(alloc.dtype)))
    assert sorted(in_names) == ["wsl", "xin"] and out_names == ["out"], (
        in_names, out_names)
    all_in = in_names + out_names
    if partition_name:
        all_in.append(partition_name)

    def _body2(xarg, warg, oarg):
        by = {"xin": xarg, "wsl": warg}
        operands = [by[n] for n in in_names] + [oarg]
        if partition_name:
            operands.append(bass2jax.partition_id_tensor())
        outs = bass2jax._bass_exec_p.bind(
            *operands, out_avals=tuple(out_avals), in_names=tuple(all_in),
            out_names=tuple(out_names), lowering_input_output_aliases=(),
            sim_require_finite=True, sim_require_nnan=True, nc=nc)
        return outs[0]

    # The 8 cores are independent (core = batch index), so the launch is
    # split into sub-mesh groups; a later group's upload overlaps an
    # earlier group's execute/readback on the (partially duplex) tunnel.
    sizes = GROUP_SIZES
    assert sum(sizes) == NCORES
    ngroups = len(sizes)
    oshape = tuple(out_avals[0].shape)
    groups = []
    dev0 = 0
    for gsz in sizes:
        devices = jax.devices()[dev0:dev0 + gsz]
        dev0 += gsz
        mesh = Mesh(np.asarray(devices), ("core",))
        shard = NamedSharding(mesh, PartitionSpec("core"))
        # No donation: the NEFF writes the custom call's result buffer, not
        # the "out"-slot operand, so one cached zeros buffer serves forever.
        sharded = jax.jit(
            shard_map(_body2, mesh=mesh,
                      in_specs=(PartitionSpec("core"),) * 3,
                      out_specs=PartitionSpec("core"), check_rep=False),
            keep_unused=True)
        dz = jax.device_put(
            np.zeros((gsz * oshape[0], *oshape[1:]), mybir.dt.np(I8)), shard)
        groups.append((shard, sharded, dz))

    if ngroups > 1 and "pool" not in _CACHE:
        _CACHE["pool"] = ThreadPoolExecutor(max_workers=ngroups)

    def run(x, W, b, res):
        wkey = hashlib.blake2b(
            np.asarray(W).tobytes() + np.asarray(b).tobytes(),
            digest_size=16).digest()
        if _CACHE.get("wkey") != wkey:
            wsl = make_w_slab(W, b)
            _CACHE["dw"] = [
                jax.device_put(
                    wsl[sum(sizes[:g]) * WSLAB:sum(sizes[:g + 1]) * WSLAB],
                    groups[g][0])
                for g in range(ngroups)]
            _CACHE["wkey"] = wkey

        def one_group(g):
            # slice+cast, upload, launch, gather, dequant — all inside the
            # worker so host work overlaps other groups' transfers
            shard, sharded, dz = groups[g]
            c0, c1 = sum(sizes[:g]), sum(sizes[:g + 1])
            xg = x[:, :, :, c0:c1, :].transpose(3, 0, 4, 1, 2).astype(
                NP_F16).reshape(sizes[g] * 128, 1024)
            dx = jax.device_put(xg, shard)
            o = sharded(dx, _CACHE["dw"][g], dz)
            if ASYNC_GATHER:
                o.copy_to_host_async()
            np.multiply(np.asarray(o), np.float32(1.0 / OSCALE),
                        dtype=np.float32, out=res[c0 * 1024:c1 * 1024])

        if ngroups == 1:
            one_group(0)
        else:
            list(_CACHE["pool"].map(one_group, range(ngroups)))

    _CACHE["runner"] = run
    return run


def kernel(x, W, b):
    run = _get_runner()
    x = np.asarray(x)
    res = np.empty((NCORES * H * W_, O), np.float32)
    run(x, W, b, res)
    return res.reshape(NCORES, H, W_, NC, DC)


# revision 26
# speedup vs baseline: 1.0782x; 1.0782x over previous
"""ConvCapsule Trainium2 kernel.

Full inputs -> 8-way batch-parallel (over output batch b) -> full output.

Math (per core, b = core id):
  img j in 0..7:  votes[j] = conv3x3_SAME(x[j, :, :, b, :], W)  -> [32,32,256]
  preact1 = (1/16) * sum_j votes[j] + bias          (softmax of zero logits = 1/16)
  act1    = squash(preact1)   [squash over dc groups of 16]
  logits[j, s, nc] = sum_dc votes[j][s, nc, dc] * act1[s, nc, dc]
  route   = softmax(logits over nc)
  preact2 = sum_j route[j] * votes[j] + bias
  out     = squash(preact2)

The end-to-end call is dominated by the axon tunnel: ~70-80 ms fixed
round-trip latency on any awaited op, ~30 MB/s up / ~22 MB/s down
(window-limited, single shared connection, partially duplex). The design
minimizes bytes moved and overlaps the rest:
  - per-core x slice ships as fp16 [128, 1024] (img*ch channel-major); the
    im2col expansion happens ON DEVICE via strided SBUF->SBUF DMAs instead
    of shipping a 4 MB/core S tensor.
  - the packed conv weight tables (identical on every core) are uploaded
    once and cached on device, keyed by content hash of (W, b).
  - output is int8 with fixed scale 127 (squash output is always in
    (-1, 1)), dequantized on host; halves the readback vs fp16. The DVE
    f32->int8 conversion rounds to nearest (max quant err 1/(2*127)).
  - the jax.jit(shard_map) executables are built once and cached. No
    donation: the NEFF writes the custom call's result buffer, so one
    cached device-side zeros buffer serves the "out" operand forever.
  - cores are independent (core = batch index), so the launch is split
    into GROUP_SIZES sub-mesh groups driven by worker threads; a later
    group's upload and host prep overlap an earlier group's execute and
    readback, amortizing the fixed tunnel latency to ~one quantum/call.

Device mapping (per core):
  - conv as 2 accumulated matmuls (K=96/97 + K=48) per 128-pixel chunk per
    image, reading 6 w/h-shifted channel groups from the on-device-built S
    tile (zero padded, ones row for fused bias).
  - preact1 via duplicate matmuls with W/16-scaled weights accumulating in
    PSUM.
  - routing on DVE/ACT/GPSIMD; squash factor applied after the grouped
    reduce (linearity); output quantization scale folded into the final
    squash factor.
"""

import hashlib
from concurrent.futures import ThreadPoolExecutor

import numpy as np

import jax
import jax.numpy as jnp
from jax.sharding import Mesh, NamedSharding, PartitionSpec
from jax.experimental.shard_map import shard_map

import concourse.bacc as bacc
import concourse.tile as tile
from concourse import mybir
from concourse import bass2jax

F32 = mybir.dt.float32
F16 = mybir.dt.float16
I8 = mybir.dt.int8
NP_F16 = np.float16
AF = mybir.ActivationFunctionType
OP = mybir.AluOpType

B, H, W_, NIN, DIN = 8, 32, 32, 8, 16
NC, DC = 16, 16
O = NC * DC           # 256 out channels
SF = 36 * 32          # S free dim: 34 zero-padded rows of 32, + 2 rows tail
EPS = 1e-9
NCHUNK = 8            # spatial chunks of 128 pixels (4 rows)
NCORES = 8
GROUP_SIZES = (4, 4)    # cores per sub-mesh launch group (pipelining)
ASYNC_GATHER = True     # issue copy_to_host_async right after launch
OSCALE = 127.0        # int8 output quantization scale
GPSIMD_DMULTS = 4     # how many of the 8 route*votes products go to GPSIMD
GPSIMD_BMULT = True   # B-product on gpsimd

# packed weight rows in the 256-wide view: wc96 | wc48 | wc96s | wc48s | b
WR96, WR48, WR96S, WR48S, WRB = 0, 96, 144, 241, 289
WROWS = 290                          # total packed rows
WSLAB = (WROWS * O + 1023) // 1024   # 73 rows of 1024
SHIFTS = [(-1, -1), (-1, 0), (-1, 1), (0, -1), (0, 0), (0, 1)]

_CACHE = {}


def build_module():
    nc = bacc.Bacc("TRN2", target_bir_lowering=False, debug=False)

    xin = nc.dram_tensor("xin", [128, 1024], F16, kind="ExternalInput")
    wsl = nc.dram_tensor("wsl", [WSLAB, 1024], F16, kind="ExternalInput")
    out = nc.dram_tensor("out", [H * W_, O], I8, kind="ExternalOutput")
    # 256-wide view of the packed weight slab
    wv = wsl.ap().rearrange("p (q r) -> (p q) r", r=O)

    with tile.TileContext(nc) as tc:
        with (
            tc.tile_pool(name="const", bufs=1) as constp,
            tc.tile_pool(name="simg", bufs=1) as sp,
            tc.tile_pool(name="psum", bufs=1, space="PSUM") as pp,
            tc.tile_pool(name="work", bufs=2) as wp,
            tc.tile_pool(name="small", bufs=2) as smp,
        ):
            # ---- persistent loads ----
            w96 = constp.tile([96, O], F16)
            w48 = constp.tile([48, O], F16)
            w96s = constp.tile([97, O], F16)
            w48s = constp.tile([48, O], F16)
            brow = constp.tile([1, O], F16)
            nc.sync.dma_start(w96[:], wv[WR96:WR96 + 96])
            nc.sync.dma_start(w48[:], wv[WR48:WR48 + 48])
            nc.sync.dma_start(w96s[:], wv[WR96S:WR96S + 97])
            nc.sync.dma_start(w48s[:], wv[WR48S:WR48S + 48])
            nc.sync.dma_start(brow[:], wv[WRB:WRB + 1])

            xall = sp.tile([128, 1024], F16, name="xall")
            nc.sync.dma_start(xall[:], xin.ap())

            # bias tile [128, O] = ones[128]^T (x) brow, via PE broadcast
            ones1 = constp.tile([1, 128], F16)
            nc.vector.memset(ones1[:], 1.0)
            ps_b = pp.tile([128, O], F32, tag="psb", bufs=1)
            nc.tensor.matmul(ps_b[:], ones1[:], brow[:], start=True, stop=True)
            bias = constp.tile([128, O], F32)
            nc.scalar.copy(bias[:], ps_b[:])

            # ---- on-device im2col: S[j][16g+ch, r*32+w] = xpad[j,ch,r-1+dh,w+dw]
            s_tiles = []
            for j in range(NIN):
                st = sp.tile([97, SF], F16, name=f"s{j}")
                nc.vector.memset(st[0:96, :], 0.0)
                nc.gpsimd.memset(st[96:97, :], 1.0)
                s_tiles.append(st)
            xv = xall[:].rearrange("p (h w) -> p h w", w=32)
            for j in range(NIN):
                sv = s_tiles[j][:].rearrange("p (r w) -> p r w", w=32)
                for g, (dh, dw) in enumerate(SHIFTS):
                    rlo, rhi = max(0, 1 - dh), min(34, 33 - dh)
                    wlo, whi = max(0, -dw), min(32, 32 - dw)
                    nc.sync.dma_start(
                        sv[16 * g:16 * g + 16, rlo:rhi, wlo:whi],
                        xv[j * 16:(j + 1) * 16,
                           rlo - 1 + dh:rhi - 1 + dh, wlo + dw:whi + dw])

            for c in range(NCHUNK):
                h0 = 4 * c
                # ---------------- conv ----------------
                ps_votes = pp.tile([128, NIN * O], F32, tag="psv", bufs=1)
                ps_pre1 = pp.tile([128, O], F32, tag="psp", bufs=1)
                p0 = (h0 + 1) * 32
                for j in range(NIN):
                    st = s_tiles[j]
                    l96 = st[0:96, p0:p0 + 128]
                    l97 = st[0:97, p0:p0 + 128]
                    l48 = st[0:48, p0 + 64:p0 + 192]
                    vslice = ps_votes[:, j * O:(j + 1) * O]
                    nc.tensor.matmul(vslice, l96, w96[:], start=True, stop=False,
                                     skip_group_check=True)
                    if j == 0:
                        nc.tensor.matmul(ps_pre1[:], l97, w96s[:],
                                         start=True, stop=False,
                                         skip_group_check=True)
                    else:
                        nc.tensor.matmul(ps_pre1[:], l96, w96s[0:96],
                                         start=False, stop=False,
                                         skip_group_check=True)
                    nc.tensor.matmul(vslice, l48, w48[:], start=False, stop=True,
                                     skip_group_check=True)
                    nc.tensor.matmul(ps_pre1[:], l48[0:48], w48s[:],
                                     start=False, stop=(j == NIN - 1),
                                     skip_group_check=True)

                # ---------------- evict ----------------
                votes = wp.tile([128, NIN * O], F32, tag="votes")
                pre1 = smp.tile([128, O], F32, tag="pre1")
                nc.scalar.copy(votes[:], ps_votes[:])
                nc.scalar.copy(pre1[:], ps_pre1[:])

                # ---------------- squash factor f1 from preact1 ----------------
                sqel1 = smp.tile([128, O], F32, tag="sqel1")
                nc.scalar.square(sqel1[:], pre1[:])
                sq1 = smp.tile([128, NC], F32, tag="sq1")
                nc.vector.reduce_sum(
                    sq1[:], sqel1[:].rearrange("p (n d) -> p n d", d=DC),
                    axis=mybir.AxisListType.X)
                f1 = _squash_factor(nc, smp, sq1, "1")

                # ---------------- logits ----------------
                pall = wp.tile([128, NIN * O], F32, tag="pall")
                v3 = votes[:].rearrange("p (j o) -> p j o", j=NIN)
                p1b = pre1[:].unsqueeze(1).broadcast_to([128, NIN, O])
                eng_b = nc.gpsimd if GPSIMD_BMULT else nc.vector
                eng_b.tensor_tensor(
                    pall[:].rearrange("p (j o) -> p j o", j=NIN), v3, p1b, op=OP.mult)
                lg = smp.tile([128, NIN * NC], F32, tag="lg")
                nc.vector.reduce_sum(
                    lg[:], pall[:].rearrange("p (j n d) -> p j n d", n=NC, d=DC),
                    axis=mybir.AxisListType.X)
                logits = smp.tile([128, NIN * NC], F32, tag="logits")
                f1b = f1[:].unsqueeze(1).broadcast_to([128, NIN, NC])
                nc.vector.tensor_tensor(
                    logits[:].rearrange("p (j n) -> p j n", j=NIN),
                    lg[:].rearrange("p (j n) -> p j n", j=NIN), f1b, op=OP.mult)

                # ---------------- softmax over nc ----------------
                ee = smp.tile([128, NIN * NC], F32, tag="ee")
                nc.scalar.activation(ee[:], logits[:], AF.Exp)
                den = smp.tile([128, NIN], F32, tag="den")
                nc.vector.reduce_sum(
                    den[:], ee[:].rearrange("p (j n) -> p j n", j=NIN),
                    axis=mybir.AxisListType.X)
                rcp = smp.tile([128, NIN], F32, tag="rcp")
                nc.vector.reciprocal(rcp[:], den[:])

                # ---------------- preact2 = sum_j route*votes + b ----------------
                route = smp.tile([128, NIN * NC], F32, tag="route")
                rcpb = rcp[:].unsqueeze(2).broadcast_to([128, NIN, NC])
                nc.vector.tensor_tensor(
                    route[:].rearrange("p (j n) -> p j n", j=NIN),
                    ee[:].rearrange("p (j n) -> p j n", j=NIN), rcpb, op=OP.mult)
                p2 = wp.tile([128, NIN * O], F32, tag="p2")
                for j in range(NIN):
                    rj = route[:, j * NC:(j + 1) * NC]
                    rjb = rj.unsqueeze(2).broadcast_to([128, NC, DC])
                    eng = nc.gpsimd if j < GPSIMD_DMULTS else nc.vector
                    eng.tensor_tensor(
                        p2[:, j * O:(j + 1) * O].rearrange("p (n d) -> p n d", n=NC),
                        votes[:, j * O:(j + 1) * O].rearrange("p (n d) -> p n d", n=NC),
                        rjb, op=OP.mult)
                pre2 = smp.tile([128, O], F32, tag="pre2")
                nc.vector.reduce_sum(
                    pre2[:],
                    p2[:].rearrange("p (j n d) -> p n d j", j=NIN, n=NC),
                    axis=mybir.AxisListType.X)
                pre2b = smp.tile([128, O], F32, tag="pre2b")
                nc.vector.tensor_tensor(pre2b[:], pre2[:], bias[:], op=OP.add)

                # ---------------- final squash, quantized to int8 ----------------
                sqel2 = smp.tile([128, O], F32, tag="sqel2")
                nc.scalar.square(sqel2[:], pre2b[:])
                sq2 = smp.tile([128, NC], F32, tag="sq2")
                nc.vector.reduce_sum(
                    sq2[:], sqel2[:].rearrange("p (n d) -> p n d", d=DC),
                    axis=mybir.AxisListType.X)
                f2 = _squash_factor(nc, smp, sq2, "2")
                f2s = smp.tile([128, NC], F32, tag="f2s")
                nc.vector.tensor_scalar_mul(f2s[:], f2[:], OSCALE)
                act2 = wp.tile([128, O], I8, tag="act2")
                f2b = f2s[:].unsqueeze(2).broadcast_to([128, NC, DC])
                nc.vector.tensor_tensor(
                    act2[:].rearrange("p (n d) -> p n d", n=NC),
                    pre2b[:].rearrange("p (n d) -> p n d", n=NC), f2b, op=OP.mult)

                nc.sync.dma_start(out.ap()[c * 128:(c + 1) * 128], act2[:])

    nc.compile()
    return nc


def _squash_factor(nc, pool, sq, tag):
    """f = sq / ((1+sq) * sqrt(sq+EPS)), shape [128, NC]."""
    sqe = pool.tile([128, NC], F32, name=f"sqe{tag}", tag=f"sqe{tag}")
    nc.vector.tensor_scalar_add(sqe[:], sq[:], EPS)
    rt = pool.tile([128, NC], F32, name=f"rt{tag}", tag=f"rt{tag}")
    nc.scalar.activation(rt[:], sqe[:], AF.Sqrt)
    u = pool.tile([128, NC], F32, name=f"u{tag}", tag=f"u{tag}")
    nc.vector.tensor_scalar_add(u[:], sq[:], 1.0)
    w = pool.tile([128, NC], F32, name=f"w{tag}", tag=f"w{tag}")
    nc.vector.tensor_tensor(w[:], u[:], rt[:], op=OP.mult)
    vr = pool.tile([128, NC], F32, name=f"vr{tag}", tag=f"vr{tag}")
    nc.vector.reciprocal(vr[:], w[:])
    f = pool.tile([128, NC], F32, name=f"f{tag}", tag=f"f{tag}")
    nc.vector.tensor_tensor(f[:], sq[:], vr[:], op=OP.mult)
    return f


def make_x_input(x):
    """[core, img, ch, h, w] fp16, flattened to the global [8*128, 1024]."""
    x = np.asarray(x)
    # transpose view + astype = one-pass permute-and-cast
    return x.transpose(3, 0, 4, 1, 2).astype(NP_F16).reshape(
        NCORES * 128, 1024)


def make_w_slab(W, b):
    """Packed weight tables, tiled per core: [8*73, 1024] fp16."""
    W = np.asarray(W, dtype=np.float32)
    b = np.asarray(b, dtype=np.float32)
    wpack = np.zeros((WROWS, O), np.float32)
    for g in range(6):
        kh, kw = (0, g) if g < 3 else (1, g - 3)
        wpack[WR96 + 16 * g:WR96 + 16 * g + 16] = W[kh, kw]
    for g in range(3):
        wpack[WR48 + 16 * g:WR48 + 16 * g + 16] = W[2, g]
    bflat = b.reshape(O)
    wpack[WR96S:WR96S + 96] = wpack[WR96:WR96 + 96] / 16.0
    wpack[WR96S + 96] = bflat
    wpack[WR48S:WR48S + 48] = wpack[WR48:WR48 + 48] / 16.0
    wpack[WRB] = bflat

    wslab = np.zeros((WSLAB, 1024), NP_F16)
    wslab.reshape(-1)[:WROWS * O] = wpack.reshape(-1).astype(NP_F16)
    return np.tile(wslab, (NCORES, 1))


def _get_runner():
    if "runner" in _CACHE:
        return _CACHE["runner"]

    nc = _CACHE.get("nc")
    if nc is None:
        nc = _CACHE["nc"] = build_module()

    bass2jax.install_neuronx_cc_hook()
    partition_name = nc.partition_id_tensor.name if nc.partition_id_tensor else None

    in_names, out_names, out_avals = [], [], []
    for alloc in nc.m.functions[0].allocations:
        if not isinstance(alloc, mybir.MemoryLocationSet):
            continue
        name = alloc.memorylocations[0].name
        if alloc.kind == "ExternalInput":
            if name != partition_name:
                in_names.append(name)
        elif alloc.kind == "ExternalOutput":
            out_names.append(name)
            out_avals.append(jax.core.ShapedArray(
                tuple(alloc.tensor_shape), mybir.dt.np(alloc.dtype)))
    assert sorted(in_names) == ["wsl", "xin"] and out_names == ["out"], (
        in_names, out_names)
    all_in = in_names + out_names
    if partition_name:
        all_in.append(partition_name)

    def _body2(xarg, warg, oarg):
        by = {"xin": xarg, "wsl": warg}
        operands = [by[n] for n in in_names] + [oarg]
        if partition_name:
            operands.append(bass2jax.partition_id_tensor())
        outs = bass2jax._bass_exec_p.bind(
            *operands, out_avals=tuple(out_avals), in_names=tuple(all_in),
            out_names=tuple(out_names), lowering_input_output_aliases=(),
            sim_require_finite=True, sim_require_nnan=True, nc=nc)
        return outs[0]

    # The 8 cores are independent (core = batch index), so the launch is
    # split into sub-mesh groups; a later group's upload overlaps an
    # earlier group's execute/readback on the (partially duplex) tunnel.
    sizes = GROUP_SIZES
    assert sum(sizes) == NCORES
    ngroups = len(sizes)
    oshape = tuple(out_avals[0].shape)
    groups = []
    dev0 = 0
    for gsz in sizes:
        devices = jax.devices()[dev0:dev0 + gsz]
        dev0 += gsz
        mesh = Mesh(np.asarray(devices), ("core",))
        shard = NamedSharding(mesh, PartitionSpec("core"))
        # No donation: the NEFF writes the custom call's result buffer, not
        # the "out"-slot operand, so one cached zeros buffer serves forever.
        sharded = jax.jit(
            shard_map(_body2, mesh=mesh,
                      in_specs=(PartitionSpec("core"),) * 3,
                      out_specs=PartitionSpec("core"), check_rep=False),
            keep_unused=True)
        dz = jax.device_put(
            np.zeros((gsz * oshape[0], *oshape[1:]), mybir.dt.np(I8)), shard)
        groups.append((shard, sharded, dz))

    if ngroups > 1 and "pool" not in _CACHE:
        _CACHE["pool"] = ThreadPoolExecutor(max_workers=ngroups)

    def run(x, W, b, res):
        wkey = hashlib.blake2b(
            np.asarray(W).tobytes() + np.asarray(b).tobytes(),
            digest_size=16).digest()
        if _CACHE.get("wkey") != wkey:
            wsl = make_w_slab(W, b)
            _CACHE["dw"] = [
                jax.device_put(
                    wsl[sum(sizes[:g]) * WSLAB:sum(sizes[:g + 1]) * WSLAB],
                    groups[g][0])
                for g in range(ngroups)]
            _CACHE["wkey"] = wkey

        def one_group(g):
            # slice+cast, upload, launch, gather, dequant — all inside the
            # worker so host work overlaps other groups' transfers
            shard, sharded, dz = groups[g]
            c0, c1 = sum(sizes[:g]), sum(sizes[:g + 1])
            xg = x[:, :, :, c0:c1, :].transpose(3, 0, 4, 1, 2).astype(
                NP_F16).reshape(sizes[g] * 128, 1024)
            dx = jax.device_put(xg, shard)
            o = sharded(dx, _CACHE["dw"][g], dz)
            if ASYNC_GATHER:
                o.copy_to_host_async()
            np.multiply(np.asarray(o), np.float32(1.0 / OSCALE),
                        dtype=np.float32, out=res[c0 * 1024:c1 * 1024])

        if ngroups == 1:
            one_group(0)
        else:
            list(_CACHE["pool"].map(one_group, range(ngroups)))

    _CACHE["runner"] = run
    return run


def kernel(x, W, b):
    run = _get_runner()
    x = np.asarray(x)
    res = np.empty((NCORES * H * W_, O), np.float32)
    run(x, W, b, res)
    return res.reshape(NCORES, H, W_, NC, DC)
